# revision 6
# baseline (speedup 1.0000x reference)
"""Multi-head attention (B=4, S=2048, D=1024, H=16, d_k=64) on 8 TRN2 NeuronCores.

Sharding: batch (4) x head-group (2) = 8 cores. Each core computes, for its
batch b and its 8 heads:
  - column-parallel Q/K/V projections (512 output features),
  - full attention for those heads,
  - row-parallel partial output projection (contribution of its 512 features
    to all 1024 output features).
The host sums the two partial outputs per batch (Megatron row-parallel
reduction done on host) and adds the output bias.

On-chip layout (per core, f32 storage; matmuls run in float32r = fp32 with an
11-bit mantissa at full PE speed):
  Q^T, K^T : [512, 2048]  features on partitions -> K=64 scores matmuls,
                          row-packed 2 heads per 128-partition tile
  V        : [2048, 520]  8 heads x (64 values + a ones column); the ones
                          column makes the attnV matmul emit the softmax
                          denominator for free as PSUM row 64
  scores^T : [128 j, 2x512 i] PSUM; exp on ScalarE with fused 1/8 scale
  y^T      : [512, 2048]  normalized attention outputs -> out-projection

Schedule: dense PE pre-phase (K-proj, V-proj, Q-proj for the first query
chunk), then an exp-rate-limited attention pipeline (scores run 2 j-steps
ahead of attnV) with the remaining Q-proj and the output projection injected
into the per-j PE slack so ScalarE never starves.
"""

import sys
from collections import deque

for _p in ("/root/.axon_site/_ro/trn_rl_repo", "/opt/trn_rl_repo"):
    if _p not in sys.path:
        sys.path.append(_p)

import numpy as np

import concourse.bass as bass
import concourse.mybir as mybir
import concourse.tile as tile
from concourse import bacc
from concourse.bass_utils import run_bass_kernel_spmd

F32R = mybir.dt.float32r
F32 = mybir.dt.float32
EXP = mybir.ActivationFunctionType.Exp

B, S, D = 4, 2048, 1024
H, DK = 16, 64
HG = 8  # heads per core
MLOC = HG * DK  # 512 features per core
NC_ = 8
ICN, ICS = 4, 512  # query chunks
JTN, JTS = 16, 128  # key tiles
KTN = 8  # contraction tiles over D
MTN = 4  # head-pair tiles over MLOC
VW = HG * (DK + 1)  # 520: V row width incl. ones columns
HW_ = VW // 2  # 260, fits one PSUM bank
SCALE = 1.0 / 8.0  # 1/sqrt(DK)

_CACHE: dict = {}


def _round_f32r(a: np.ndarray) -> np.ndarray:
    """Round fp32 to the fp32r grid (11-bit mantissa; low 12 bits zero)."""
    b = np.ascontiguousarray(a, dtype=np.float32).view(np.uint32)
    b = (b + 0x800) & 0xFFFFF000
    return b.view(np.float32)


def _build():
    nc = bacc.Bacc("TRN2", target_bir_lowering=False, debug=False, num_devices=NC_)

    d_xq = nc.dram_tensor("xq_t", [D, S], F32R, kind="ExternalInput").ap()
    d_xk = nc.dram_tensor("xk_t", [D, S], F32R, kind="ExternalInput").ap()
    d_xv = nc.dram_tensor("xv_t", [D, S], F32R, kind="ExternalInput").ap()
    d_wq = nc.dram_tensor("wq_t", [D, MLOC], F32R, kind="ExternalInput").ap()
    d_wk = nc.dram_tensor("wk_t", [D, MLOC], F32R, kind="ExternalInput").ap()
    d_wv = nc.dram_tensor("wv_a", [D, VW], F32R, kind="ExternalInput").ap()
    d_bv = nc.dram_tensor("bv_a", [1, VW], F32R, kind="ExternalInput").ap()
    d_wo = nc.dram_tensor("wo_t", [MLOC, D], F32R, kind="ExternalInput").ap()
    d_bq = nc.dram_tensor("bq_r", [128, MTN], F32, kind="ExternalInput").ap()
    d_bk = nc.dram_tensor("bk_r", [128, MTN], F32, kind="ExternalInput").ap()
    d_on = nc.dram_tensor("ones_r", [1, 128], F32R, kind="ExternalInput").ap()
    d_out = nc.dram_tensor("out_t", [D, S], F32, kind="ExternalOutput").ap()

    with tile.TileContext(nc) as tc:
        with (
            tc.tile_pool(name="persist", bufs=1) as pp,
            tc.tile_pool(name="xstream", bufs=1) as xsp,
            tc.tile_pool(name="work", bufs=1) as wk_pool,
            tc.tile_pool(name="taskp", bufs=1, space="PSUM") as tkp,
        ):
            # ---- persistent SBUF tensors ----
            wv_sb = pp.tile([128, KTN, VW], F32R, tag="wv", name="wv_sb")
            nc.sync.dma_start(wv_sb[:], d_wv.rearrange("(k p) m -> p k m", p=128))
            wo_sb = pp.tile([128, MTN, D], F32R, tag="wo", name="wo_sb")
            nc.sync.dma_start(wo_sb[:], d_wo.rearrange("(k p) m -> p k m", p=128))
            wq_sb = pp.tile([128, KTN, MLOC], F32R, tag="wq", name="wq_sb")
            nc.sync.dma_start(wq_sb[:], d_wq.rearrange("(k p) m -> p k m", p=128))
            bv_sb = pp.tile([1, VW], F32R, tag="bv", name="bv_sb")
            nc.sync.dma_start(bv_sb[:], d_bv[:])
            on_sb = pp.tile([1, 128], F32R, tag="ones", name="on_sb")
            nc.sync.dma_start(on_sb[:], d_on[:])
            bq_sb = pp.tile([128, MTN], F32, tag="bq", name="bq_sb")
            nc.sync.dma_start(bq_sb[:], d_bq[:])
            bk_sb = pp.tile([128, MTN], F32, tag="bk", name="bk_sb")
            nc.sync.dma_start(bk_sb[:], d_bk[:])

            qt_ic: dict = {}
            kt = [
                pp.tile([128, S], F32R, tag=f"kt{m}", name=f"kt{m}")
                for m in range(MTN)
            ]
            v_sb = [
                pp.tile([128, VW], F32R, tag=f"v{st}", name=f"v{st}")
                for st in range(JTN)
            ]

            def load_x(xdram, ic):
                """8 k-tiles [128, 512] of one x^T query chunk."""
                xts = []
                for k in range(KTN):
                    xt = xsp.tile([128, ICS], F32R, tag="x", bufs=10, name="xt")
                    nc.sync.dma_start(
                        xt[:],
                        xdram[k * 128 : (k + 1) * 128, ic * ICS : (ic + 1) * ICS],
                    )
                    xts.append(xt)
                return xts

            def proj_mm(dest_ap, w_sb, b_sb, xts, m, psum_pool, psum_bufs):
                ps = psum_pool.tile(
                    [128, ICS], F32, tag=psum_pool.name, bufs=psum_bufs, name="ps"
                )
                for k in range(KTN):
                    nc.tensor.matmul(
                        ps[:],
                        w_sb[:, k, m * 128 : (m + 1) * 128],
                        xts[k][:],
                        start=(k == 0),
                        stop=(k == KTN - 1),
                    )
                nc.vector.tensor_scalar_add(dest_ap, ps[:], b_sb[:, m : m + 1])

            def q_proj(ic, psum_pool, psum_bufs):
                xts = load_x(d_xq, ic)
                for m in range(MTN):
                    qtile = wk_pool.tile(
                        [128, ICS], F32R, tag=f"q{m}", bufs=2, name=f"q{m}"
                    )
                    qt_ic[(ic, m)] = qtile
                    proj_mm(qtile[:], wq_sb, bq_sb, xts, m, psum_pool, psum_bufs)

            # ================= pre-phase (dense PE work) =================
            with (
                tc.tile_pool(name="wkk", bufs=1) as wkk,
                tc.tile_pool(name="prep", bufs=1, space="PSUM") as prep,
            ):
                wk_sb = wkk.tile([128, KTN, MLOC], F32R, tag="wk", name="wk_sb")
                nc.sync.dma_start(wk_sb[:], d_wk.rearrange("(k p) m -> p k m", p=128))

                # K projection
                for ic in range(ICN):
                    xts = load_x(d_xk, ic)
                    for m in range(MTN):
                        proj_mm(
                            kt[m][:, ic * ICS : (ic + 1) * ICS],
                            wk_sb, bk_sb, xts, m, prep, 3,
                        )

                # V projection: x^T tiles stationary, ones row adds bias and
                # seeds the denominator columns
                for stg in range(4):
                    xvg = load_x(d_xv, stg)  # [128, 512] spanning 4 s-tiles
                    for sti in range(4):
                        st = stg * 4 + sti
                        ssl = slice(sti * 128, (sti + 1) * 128)
                        for hf in range(2):
                            pv = prep.tile(
                                [128, HW_], F32, tag="pv", bufs=3, name="pv"
                            )
                            nc.tensor.matmul(
                                pv[:],
                                on_sb[0:1, :],
                                bv_sb[0:1, hf * HW_ : (hf + 1) * HW_],
                                start=True,
                                stop=False,
                            )
                            for k in range(KTN):
                                nc.tensor.matmul(
                                    pv[:],
                                    xvg[k][:, ssl],
                                    wv_sb[:, k, hf * HW_ : (hf + 1) * HW_],
                                    start=False,
                                    stop=(k == KTN - 1),
                                )
                            nc.vector.tensor_copy(
                                v_sb[st][:, hf * HW_ : (hf + 1) * HW_], pv[:]
                            )

                # Q projection for the first query chunk
                q_proj(0, prep, 3)

            # ================= attention phase =================
            # Background PE tasks (Q-proj ic 1..3, out-proj per finished ic)
            # popped into the per-j slack of the exp-rate-limited pipeline.
            tasks = deque()
            for ic in range(1, ICN):
                tasks.append(lambda ic=ic: q_proj(ic, tkp, 2))

            yts_by_ic = {}

            def o_task(ic, fo):
                po = tkp.tile([128, ICS], F32, tag="taskp", bufs=2, name="po")
                for m in range(MTN):
                    nc.tensor.matmul(
                        po[:],
                        wo_sb[:, m, fo * 128 : (fo + 1) * 128],
                        yts_by_ic[ic][m][:],
                        start=(m == 0),
                        stop=(m == MTN - 1),
                    )
                ot = wk_pool.tile([128, ICS], F32, tag="ot", bufs=2, name="ot")
                nc.vector.tensor_copy(ot[:], po[:])
                nc.sync.dma_start(
                    d_out[fo * 128 : (fo + 1) * 128, ic * ICS : (ic + 1) * ICS],
                    ot[:],
                )

            with (
                tc.tile_pool(name="scp", bufs=1, space="PSUM") as scp,
                tc.tile_pool(name="avp", bufs=1, space="PSUM") as avp,
            ):
                pop_budget = 0.0
                for ic in range(ICN):
                    isl = slice(ic * ICS, (ic + 1) * ICS)
                    yts = []
                    for mt in range(MTN):
                        pva = avp.tile([65, ICS], F32, tag="pv", bufs=2, name="pva")
                        pvb = avp.tile([65, ICS], F32, tag="pv", bufs=2, name="pvb")

                        sc_tiles = {}

                        qcur = qt_ic[(ic, mt)]

                        def emit_scores(j, mt=mt, qcur=qcur, sc_tiles=sc_tiles):
                            jsl = slice(j * JTS, (j + 1) * JTS)
                            sc = scp.tile(
                                [128, 2 * ICS], F32, tag="sc", bufs=2, name="sc"
                            )
                            nc.tensor.matmul(
                                sc[:, 0:ICS],
                                kt[mt][0:64, jsl],
                                qcur[0:64, :],
                                start=True,
                                stop=True,
                            )
                            nc.tensor.matmul(
                                sc[:, ICS : 2 * ICS],
                                kt[mt][64:128, jsl],
                                qcur[64:128, :],
                                start=True,
                                stop=True,
                            )
                            sc_tiles[j] = sc

                        emit_scores(0)
                        emit_scores(1)
                        for j in range(JTN):
                            sc = sc_tiles.pop(j)
                            u = wk_pool.tile(
                                [128, 2 * ICS], F32R, tag="u", bufs=2, name="u"
                            )
                            nc.scalar.activation(u[:], sc[:], EXP, scale=SCALE)
                            va = v_sb[j][:, (2 * mt) * 65 : (2 * mt) * 65 + 65]
                            vb = v_sb[j][
                                :, (2 * mt + 1) * 65 : (2 * mt + 1) * 65 + 65
                            ]
                            nc.tensor.matmul(
                                pva[:], va, u[:, 0:ICS],
                                start=(j == 0), stop=(j == JTN - 1),
                            )
                            nc.tensor.matmul(
                                pvb[:], vb, u[:, ICS : 2 * ICS],
                                start=(j == 0), stop=(j == JTN - 1),
                            )
                            if j + 2 < JTN:
                                emit_scores(j + 2)
                            # ~1 background task per 3 j-steps keeps PE slack used
                            pop_budget += 1.0 / 3.0
                            if pop_budget >= 1.0 and tasks:
                                pop_budget -= 1.0
                                tasks.popleft()()

                        # normalization: reciprocal of denominator row, DMA
                        # broadcast across 64 partitions, multiply into y^T
                        yt_t = wk_pool.tile(
                            [128, ICS], F32R, tag=f"yt{mt}", bufs=2, name=f"yt{mt}"
                        )
                        rca = wk_pool.tile([1, ICS], F32, tag="rc", bufs=2, name="rc")
                        rcb = wk_pool.tile([1, ICS], F32, tag="rc", bufs=2, name="rc2")
                        nc.vector.reciprocal(rca[:], pva[64:65, :])
                        nc.vector.reciprocal(rcb[:], pvb[64:65, :])
                        bca = wk_pool.tile([64, ICS], F32, tag="bc", bufs=2, name="bc")
                        bcb = wk_pool.tile([64, ICS], F32, tag="bc", bufs=2, name="bc2")
                        nc.sync.dma_start(
                            bca[:],
                            rca[0:1, :].unsqueeze(1).broadcast_to((1, 64, ICS)),
                        )
                        nc.sync.dma_start(
                            bcb[:],
                            rcb[0:1, :].unsqueeze(1).broadcast_to((1, 64, ICS)),
                        )
                        nc.vector.tensor_mul(yt_t[0:64, :], pva[0:64, :], bca[:])
                        nc.vector.tensor_mul(yt_t[64:128, :], pvb[0:64, :], bcb[:])
                        yts.append(yt_t)

                    yts_by_ic[ic] = yts
                    for fo in range(KTN):
                        tasks.append(lambda ic=ic, fo=fo: o_task(ic, fo))

                while tasks:
                    tasks.popleft()()

    nc.compile()
    return nc


def _get_nc():
    if "nc" not in _CACHE:
        _CACHE["nc"] = _build()
    return _CACHE["nc"]


def _prep_core(c, query, key, value, Wq, bq, Wk, bk, Wv, bv, Wo, bo):
    b, hg = c // 2, c % 2
    fsl = slice(hg * MLOC, (hg + 1) * MLOC)
    r = _round_f32r
    wv_a = np.zeros((D, VW), np.float32)
    bv_a = np.zeros((1, VW), np.float32)
    for h in range(HG):
        gh = hg * HG + h
        wv_a[:, h * 65 : h * 65 + 64] = Wv[gh * 64 : (gh + 1) * 64, :].T
        bv_a[0, h * 65 : h * 65 + 64] = bv[gh * 64 : (gh + 1) * 64]
        bv_a[0, h * 65 + 64] = 1.0
    return {
        "xq_t": r(query[b].T),
        "xk_t": r(key[b].T),
        "xv_t": r(value[b].T),
        "wq_t": r(Wq[fsl, :].T),
        "wk_t": r(Wk[fsl, :].T),
        "wv_a": r(wv_a),
        "bv_a": r(bv_a),
        "wo_t": r(Wo[:, fsl].T),
        "bq_r": np.ascontiguousarray(bq[fsl].reshape(MTN, 128).T, dtype=np.float32),
        "bk_r": np.ascontiguousarray(bk[fsl].reshape(MTN, 128).T, dtype=np.float32),
        "ones_r": np.ones((1, 128), np.float32),
    }


def kernel(query, key, value, Wq, bq, Wk, bk, Wv, bv, Wo, bo, _trace=False):
    query = np.asarray(query, dtype=np.float32)
    key = np.asarray(key, dtype=np.float32)
    value = np.asarray(value, dtype=np.float32)
    Wq, bq = np.asarray(Wq, np.float32), np.asarray(bq, np.float32)
    Wk, bk = np.asarray(Wk, np.float32), np.asarray(bk, np.float32)
    Wv, bv = np.asarray(Wv, np.float32), np.asarray(bv, np.float32)
    Wo, bo = np.asarray(Wo, np.float32), np.asarray(bo, np.float32)

    nc = _get_nc()
    in_maps = [
        _prep_core(c, query, key, value, Wq, bq, Wk, bk, Wv, bv, Wo, bo)
        for c in range(NC_)
    ]
    res = run_bass_kernel_spmd(nc, in_maps, core_ids=list(range(NC_)), trace=_trace)
    _CACHE["last_result"] = res

    out = np.empty((B, S, D), np.float32)
    for b in range(B):
        acc = res.results[2 * b]["out_t"].astype(np.float32) + res.results[
            2 * b + 1
        ]["out_t"].astype(np.float32)
        out[b] = acc.T + bo[None, :]
    return out


# revision 8
# speedup vs baseline: 1.2449x; 1.2449x over previous
"""Multi-head attention (B=4, S=2048, D=1024, H=16, d_k=64) on 8 TRN2 NeuronCores.

Sharding: batch (4) x head-group (2) = 8 cores. Each core computes, for its
batch b and its 8 heads:
  - column-parallel Q/K/V projections (512 output features),
  - full attention for those heads,
  - row-parallel partial output projection (contribution of its 512 features
    to all 1024 output features).
The host sums the two partial outputs per batch (Megatron row-parallel
reduction done on host) and adds the output bias.

On-chip layout (per core, f32 storage; matmuls run in float32r = fp32 with an
11-bit mantissa at full PE speed):
  Q^T, K^T : [512, 2048]  features on partitions -> K=64 scores matmuls,
                          row-packed 2 heads per 128-partition tile
  V        : [2048, 520]  8 heads x (64 values + a ones column); the ones
                          column makes the attnV matmul emit the softmax
                          denominator for free as PSUM row 64
  scores^T : [128 j, 2x512 i] PSUM; exp on ScalarE with fused 1/8 scale
  y^T      : [512, 2048]  normalized attention outputs -> out-projection

Schedule: dense PE pre-phase (K-proj, V-proj, Q-proj for the first query
chunk), then an exp-rate-limited attention pipeline (scores run 2 j-steps
ahead of attnV) with the remaining Q-proj and the output projection injected
into the per-j PE slack so ScalarE never starves.
"""

import sys
from collections import deque

for _p in ("/root/.axon_site/_ro/trn_rl_repo", "/opt/trn_rl_repo"):
    if _p not in sys.path:
        sys.path.append(_p)

import numpy as np

import concourse.bass as bass
import concourse.mybir as mybir
import concourse.tile as tile
from concourse import bacc
from concourse.bass_utils import run_bass_kernel_spmd

F32R = mybir.dt.float32r
F16 = mybir.dt.float16
F32 = mybir.dt.float32
EXP = mybir.ActivationFunctionType.Exp

B, S, D = 4, 2048, 1024
H, DK = 16, 64
HG = 8  # heads per core
MLOC = HG * DK  # 512 features per core
NC_ = 8
ICN, ICS = 4, 512  # query chunks
JTN, JTS = 16, 128  # key tiles
KTN = 8  # contraction tiles over D
MTN = 4  # head-pair tiles over MLOC
VW = HG * (DK + 1)  # 520: V row width incl. ones columns
HW_ = VW // 2  # 260, fits one PSUM bank
SCALE = 1.0 / 8.0  # 1/sqrt(DK)

_CACHE: dict = {}


def _round_f32r(a: np.ndarray) -> np.ndarray:
    """Round fp32 to the fp32r grid (11-bit mantissa; low 12 bits zero)."""
    b = np.ascontiguousarray(a, dtype=np.float32).view(np.uint32)
    b = (b + 0x800) & 0xFFFFF000
    return b.view(np.float32)


def _build():
    nc = bacc.Bacc("TRN2", target_bir_lowering=False, debug=False, num_devices=NC_)

    d_xq = nc.dram_tensor("xq_t", [D, S], F16, kind="ExternalInput").ap()
    d_xk = nc.dram_tensor("xk_t", [D, S], F16, kind="ExternalInput").ap()
    d_xv = nc.dram_tensor("xv_t", [D, S], F16, kind="ExternalInput").ap()
    d_wq = nc.dram_tensor("wq_t", [D, MLOC], F16, kind="ExternalInput").ap()
    d_wk = nc.dram_tensor("wk_t", [D, MLOC], F16, kind="ExternalInput").ap()
    d_wv = nc.dram_tensor("wv_a", [D, VW], F16, kind="ExternalInput").ap()
    d_bv = nc.dram_tensor("bv_a", [1, VW], F16, kind="ExternalInput").ap()
    d_wo = nc.dram_tensor("wo_t", [MLOC, D], F16, kind="ExternalInput").ap()
    d_bq = nc.dram_tensor("bq_r", [128, MTN], F32, kind="ExternalInput").ap()
    d_bk = nc.dram_tensor("bk_r", [128, MTN], F32, kind="ExternalInput").ap()
    d_on = nc.dram_tensor("ones_r", [1, 128], F16, kind="ExternalInput").ap()
    d_out = nc.dram_tensor("out_t", [D, S], F32, kind="ExternalOutput").ap()

    with tile.TileContext(nc) as tc:
        with (
            tc.tile_pool(name="persist", bufs=1) as pp,
            tc.tile_pool(name="xstream", bufs=1) as xsp,
            tc.tile_pool(name="work", bufs=1) as wk_pool,
            tc.tile_pool(name="taskp", bufs=1, space="PSUM") as tkp,
        ):
            # ---- persistent SBUF tensors ----
            wv_sb = pp.tile([128, KTN, VW], F16, tag="wv", name="wv_sb")
            nc.sync.dma_start(wv_sb[:], d_wv.rearrange("(k p) m -> p k m", p=128))
            wo_sb = pp.tile([128, MTN, D], F16, tag="wo", name="wo_sb")
            nc.sync.dma_start(wo_sb[:], d_wo.rearrange("(k p) m -> p k m", p=128))
            wq_sb = pp.tile([128, KTN, MLOC], F16, tag="wq", name="wq_sb")
            nc.sync.dma_start(wq_sb[:], d_wq.rearrange("(k p) m -> p k m", p=128))
            bv_sb = pp.tile([1, VW], F16, tag="bv", name="bv_sb")
            nc.sync.dma_start(bv_sb[:], d_bv[:])
            on_sb = pp.tile([1, 128], F16, tag="ones", name="on_sb")
            nc.sync.dma_start(on_sb[:], d_on[:])
            bq_sb = pp.tile([128, MTN], F32, tag="bq", name="bq_sb")
            nc.sync.dma_start(bq_sb[:], d_bq[:])
            bk_sb = pp.tile([128, MTN], F32, tag="bk", name="bk_sb")
            nc.sync.dma_start(bk_sb[:], d_bk[:])

            qt_ic: dict = {}
            kt = [
                pp.tile([128, S], F16, tag=f"kt{m}", name=f"kt{m}")
                for m in range(MTN)
            ]
            v_sb = [
                pp.tile([128, VW], F16, tag=f"v{st}", name=f"v{st}")
                for st in range(JTN)
            ]

            def load_x(xdram, ic):
                """8 k-tiles [128, 512] of one x^T query chunk."""
                xts = []
                for k in range(KTN):
                    xt = xsp.tile([128, ICS], F16, tag="x", bufs=10, name="xt")
                    nc.sync.dma_start(
                        xt[:],
                        xdram[k * 128 : (k + 1) * 128, ic * ICS : (ic + 1) * ICS],
                    )
                    xts.append(xt)
                return xts

            def proj_mm(dest_ap, w_sb, b_sb, xts, m, psum_pool, psum_bufs):
                ps = psum_pool.tile(
                    [128, ICS], F32, tag=psum_pool.name, bufs=psum_bufs, name="ps"
                )
                for k in range(KTN):
                    nc.tensor.matmul(
                        ps[:],
                        w_sb[:, k, m * 128 : (m + 1) * 128],
                        xts[k][:],
                        start=(k == 0),
                        stop=(k == KTN - 1),
                    )
                nc.vector.tensor_scalar_add(dest_ap, ps[:], b_sb[:, m : m + 1])

            def q_proj(ic, psum_pool, psum_bufs):
                xts = load_x(d_xq, ic)
                for m in range(MTN):
                    qtile = wk_pool.tile(
                        [128, ICS], F16, tag=f"q{m}", bufs=2, name=f"q{m}"
                    )
                    qt_ic[(ic, m)] = qtile
                    proj_mm(qtile[:], wq_sb, bq_sb, xts, m, psum_pool, psum_bufs)

            # ================= pre-phase (dense PE work) =================
            with (
                tc.tile_pool(name="wkk", bufs=1) as wkk,
                tc.tile_pool(name="prep", bufs=1, space="PSUM") as prep,
            ):
                wk_sb = wkk.tile([128, KTN, MLOC], F16, tag="wk", name="wk_sb")
                nc.sync.dma_start(wk_sb[:], d_wk.rearrange("(k p) m -> p k m", p=128))

                # K projection
                for ic in range(ICN):
                    xts = load_x(d_xk, ic)
                    for m in range(MTN):
                        proj_mm(
                            kt[m][:, ic * ICS : (ic + 1) * ICS],
                            wk_sb, bk_sb, xts, m, prep, 3,
                        )

                # V projection: x^T tiles stationary, ones row adds bias and
                # seeds the denominator columns
                for stg in range(4):
                    xvg = load_x(d_xv, stg)  # [128, 512] spanning 4 s-tiles
                    for sti in range(4):
                        st = stg * 4 + sti
                        ssl = slice(sti * 128, (sti + 1) * 128)
                        for hf in range(2):
                            pv = prep.tile(
                                [128, HW_], F32, tag="pv", bufs=3, name="pv"
                            )
                            nc.tensor.matmul(
                                pv[:],
                                on_sb[0:1, :],
                                bv_sb[0:1, hf * HW_ : (hf + 1) * HW_],
                                start=True,
                                stop=False,
                            )
                            for k in range(KTN):
                                nc.tensor.matmul(
                                    pv[:],
                                    xvg[k][:, ssl],
                                    wv_sb[:, k, hf * HW_ : (hf + 1) * HW_],
                                    start=False,
                                    stop=(k == KTN - 1),
                                )
                            nc.vector.tensor_copy(
                                v_sb[st][:, hf * HW_ : (hf + 1) * HW_], pv[:]
                            )

                # Q projection for the first query chunk
                q_proj(0, prep, 3)

            # ================= attention phase =================
            # Background PE tasks (Q-proj ic 1..3, out-proj per finished ic)
            # popped into the per-j slack of the exp-rate-limited pipeline.
            tasks = deque()
            for ic in range(1, ICN):
                tasks.append(lambda ic=ic: q_proj(ic, tkp, 2))

            yts_by_ic = {}

            def o_task(ic, fo):
                po = tkp.tile([128, ICS], F32, tag="taskp", bufs=2, name="po")
                for m in range(MTN):
                    nc.tensor.matmul(
                        po[:],
                        wo_sb[:, m, fo * 128 : (fo + 1) * 128],
                        yts_by_ic[ic][m][:],
                        start=(m == 0),
                        stop=(m == MTN - 1),
                    )
                ot = wk_pool.tile([128, ICS], F32, tag="ot", bufs=2, name="ot")
                nc.vector.tensor_copy(ot[:], po[:])
                nc.sync.dma_start(
                    d_out[fo * 128 : (fo + 1) * 128, ic * ICS : (ic + 1) * ICS],
                    ot[:],
                )

            with (
                tc.tile_pool(name="scp", bufs=1, space="PSUM") as scp,
                tc.tile_pool(name="avp", bufs=1, space="PSUM") as avp,
            ):
                pop_budget = 0.0
                for ic in range(ICN):
                    isl = slice(ic * ICS, (ic + 1) * ICS)
                    yts = []
                    for mt in range(MTN):
                        pva = avp.tile([65, ICS], F32, tag="pv", bufs=2, name="pva")
                        pvb = avp.tile([65, ICS], F32, tag="pv", bufs=2, name="pvb")

                        sc_tiles = {}

                        qcur = qt_ic[(ic, mt)]

                        def emit_scores(j, mt=mt, qcur=qcur, sc_tiles=sc_tiles):
                            jsl = slice(j * JTS, (j + 1) * JTS)
                            sc = scp.tile(
                                [128, 2 * ICS], F32, tag="sc", bufs=2, name="sc"
                            )
                            nc.tensor.matmul(
                                sc[:, 0:ICS],
                                kt[mt][0:64, jsl],
                                qcur[0:64, :],
                                start=True,
                                stop=True,
                            )
                            nc.tensor.matmul(
                                sc[:, ICS : 2 * ICS],
                                kt[mt][64:128, jsl],
                                qcur[64:128, :],
                                start=True,
                                stop=True,
                            )
                            sc_tiles[j] = sc

                        emit_scores(0)
                        emit_scores(1)
                        for j in range(JTN):
                            sc = sc_tiles.pop(j)
                            u = wk_pool.tile(
                                [128, 2 * ICS], F16, tag="u", bufs=2, name="u"
                            )
                            nc.scalar.activation(u[:], sc[:], EXP, scale=SCALE)
                            va = v_sb[j][:, (2 * mt) * 65 : (2 * mt) * 65 + 65]
                            vb = v_sb[j][
                                :, (2 * mt + 1) * 65 : (2 * mt + 1) * 65 + 65
                            ]
                            nc.tensor.matmul(
                                pva[:], va, u[:, 0:ICS],
                                start=(j == 0), stop=(j == JTN - 1),
                            )
                            nc.tensor.matmul(
                                pvb[:], vb, u[:, ICS : 2 * ICS],
                                start=(j == 0), stop=(j == JTN - 1),
                            )
                            if j + 2 < JTN:
                                emit_scores(j + 2)
                            # ~1 background task per 3 j-steps keeps PE slack used
                            pop_budget += 1.0 / 3.0
                            if pop_budget >= 1.0 and tasks:
                                pop_budget -= 1.0
                                tasks.popleft()()

                        # normalization: reciprocal of denominator row, DMA
                        # broadcast across 64 partitions, multiply into y^T
                        yt_t = wk_pool.tile(
                            [128, ICS], F16, tag=f"yt{mt}", bufs=2, name=f"yt{mt}"
                        )
                        dna = wk_pool.tile([1, ICS], F32, tag="dna", bufs=2, name="dna")
                        dnb = wk_pool.tile([1, ICS], F32, tag="dnb", bufs=2, name="dnb")
                        nc.vector.tensor_copy(dna[:], pva[64:65, :])
                        nc.vector.tensor_copy(dnb[:], pvb[64:65, :])
                        rca = wk_pool.tile([1, ICS], F32, tag="rca", bufs=2, name="rca")
                        rcb = wk_pool.tile([1, ICS], F32, tag="rcb", bufs=2, name="rcb")
                        nc.vector.reciprocal_approx_fast(rca[:], dna[:])
                        nc.vector.reciprocal_approx_fast(rcb[:], dnb[:])
                        bca = wk_pool.tile([64, ICS], F32, tag="bc", bufs=2, name="bc")
                        bcb = wk_pool.tile([64, ICS], F32, tag="bc", bufs=2, name="bc2")
                        nc.sync.dma_start(
                            bca[:],
                            rca[0:1, :].unsqueeze(1).broadcast_to((1, 64, ICS)),
                        )
                        nc.sync.dma_start(
                            bcb[:],
                            rcb[0:1, :].unsqueeze(1).broadcast_to((1, 64, ICS)),
                        )
                        nc.vector.tensor_mul(yt_t[0:64, :], pva[0:64, :], bca[:])
                        nc.vector.tensor_mul(yt_t[64:128, :], pvb[0:64, :], bcb[:])
                        yts.append(yt_t)

                    yts_by_ic[ic] = yts
                    for fo in range(KTN):
                        tasks.append(lambda ic=ic, fo=fo: o_task(ic, fo))

                while tasks:
                    tasks.popleft()()

    nc.compile()
    return nc


def _get_nc():
    if "nc" not in _CACHE:
        _CACHE["nc"] = _build()
    return _CACHE["nc"]


def _prep_core(c, query, key, value, Wq, bq, Wk, bk, Wv, bv, Wo, bo):
    b, hg = c // 2, c % 2
    fsl = slice(hg * MLOC, (hg + 1) * MLOC)
    r = lambda a: np.ascontiguousarray(a, dtype=np.float16)
    wv_a = np.zeros((D, VW), np.float32)
    bv_a = np.zeros((1, VW), np.float32)
    for h in range(HG):
        gh = hg * HG + h
        wv_a[:, h * 65 : h * 65 + 64] = Wv[gh * 64 : (gh + 1) * 64, :].T
        bv_a[0, h * 65 : h * 65 + 64] = bv[gh * 64 : (gh + 1) * 64]
        bv_a[0, h * 65 + 64] = 1.0
    return {
        "xq_t": r(query[b].T),
        "xk_t": r(key[b].T),
        "xv_t": r(value[b].T),
        "wq_t": r(Wq[fsl, :].T),
        "wk_t": r(Wk[fsl, :].T),
        "wv_a": r(wv_a),
        "bv_a": r(bv_a),
        "wo_t": r(Wo[:, fsl].T),
        "bq_r": np.ascontiguousarray(bq[fsl].reshape(MTN, 128).T, dtype=np.float32),
        "bk_r": np.ascontiguousarray(bk[fsl].reshape(MTN, 128).T, dtype=np.float32),
        "ones_r": np.ones((1, 128), np.float16),
    }


def kernel(query, key, value, Wq, bq, Wk, bk, Wv, bv, Wo, bo, _trace=False):
    query = np.asarray(query, dtype=np.float32)
    key = np.asarray(key, dtype=np.float32)
    value = np.asarray(value, dtype=np.float32)
    Wq, bq = np.asarray(Wq, np.float32), np.asarray(bq, np.float32)
    Wk, bk = np.asarray(Wk, np.float32), np.asarray(bk, np.float32)
    Wv, bv = np.asarray(Wv, np.float32), np.asarray(bv, np.float32)
    Wo, bo = np.asarray(Wo, np.float32), np.asarray(bo, np.float32)

    nc = _get_nc()
    in_maps = [
        _prep_core(c, query, key, value, Wq, bq, Wk, bk, Wv, bv, Wo, bo)
        for c in range(NC_)
    ]
    res = run_bass_kernel_spmd(nc, in_maps, core_ids=list(range(NC_)), trace=_trace)
    _CACHE["last_result"] = res

    out = np.empty((B, S, D), np.float32)
    for b in range(B):
        acc = res.results[2 * b]["out_t"].astype(np.float32) + res.results[
            2 * b + 1
        ]["out_t"].astype(np.float32)
        out[b] = acc.T + bo[None, :]
    return out


# revision 10
# speedup vs baseline: 1.4654x; 1.1771x over previous
"""Multi-head attention (B=4, S=2048, D=1024, H=16, d_k=64) on 8 TRN2 NeuronCores.

Sharding: batch (4) x head-group (2) = 8 cores. Each core computes, for its
batch b and its 8 heads:
  - column-parallel Q/K/V projections (512 output features),
  - full attention for those heads,
  - row-parallel partial output projection (contribution of its 512 features
    to all 1024 output features).
The host sums the two partial outputs per batch (Megatron row-parallel
reduction done on host) and adds the output bias.

On-chip layout (per core, f32 storage; matmuls run in float32r = fp32 with an
11-bit mantissa at full PE speed):
  Q^T, K^T : [512, 2048]  features on partitions -> K=64 scores matmuls,
                          row-packed 2 heads per 128-partition tile
  V        : [2048, 520]  8 heads x (64 values + a ones column); the ones
                          column makes the attnV matmul emit the softmax
                          denominator for free as PSUM row 64
  scores^T : [128 j, 2x512 i] PSUM; exp on ScalarE with fused 1/8 scale
  y^T      : [512, 2048]  normalized attention outputs -> out-projection

Schedule: dense PE pre-phase (K-proj, V-proj, Q-proj for the first query
chunk), then an exp-rate-limited attention pipeline (scores run 2 j-steps
ahead of attnV) with the remaining Q-proj and the output projection injected
into the per-j PE slack so ScalarE never starves.
"""

import sys
from collections import deque

for _p in ("/root/.axon_site/_ro/trn_rl_repo", "/opt/trn_rl_repo"):
    if _p not in sys.path:
        sys.path.append(_p)

import numpy as np

import concourse.bass as bass
import concourse.mybir as mybir
import concourse.tile as tile
from concourse import bacc
from concourse.bass_utils import run_bass_kernel_spmd

F32R = mybir.dt.float32r
F16 = mybir.dt.float16
F32 = mybir.dt.float32
EXP = mybir.ActivationFunctionType.Exp

B, S, D = 4, 2048, 1024
H, DK = 16, 64
HG = 8  # heads per core
MLOC = HG * DK  # 512 features per core
NC_ = 8
ICN, ICS = 4, 512  # query chunks
JTN, JTS = 16, 128  # key tiles
KTN = 8  # contraction tiles over D
MTN = 4  # head-pair tiles over MLOC
VW = HG * (DK + 1)  # 520: V row width incl. ones columns
HW_ = VW // 2  # 260, fits one PSUM bank
SCALE = 1.0 / 8.0  # 1/sqrt(DK)

_CACHE: dict = {}


def _round_f32r(a: np.ndarray) -> np.ndarray:
    """Round fp32 to the fp32r grid (11-bit mantissa; low 12 bits zero)."""
    b = np.ascontiguousarray(a, dtype=np.float32).view(np.uint32)
    b = (b + 0x800) & 0xFFFFF000
    return b.view(np.float32)


def _build():
    nc = bacc.Bacc("TRN2", target_bir_lowering=False, debug=False, num_devices=NC_)

    d_xq = nc.dram_tensor("xq_t", [D, S], F16, kind="ExternalInput").ap()
    d_xk = nc.dram_tensor("xk_t", [D, S], F16, kind="ExternalInput").ap()
    d_xv = nc.dram_tensor("xv_t", [D, S], F16, kind="ExternalInput").ap()
    d_wq = nc.dram_tensor("wq_t", [D, MLOC], F16, kind="ExternalInput").ap()
    d_wk = nc.dram_tensor("wk_t", [D, MLOC], F16, kind="ExternalInput").ap()
    d_wv = nc.dram_tensor("wv_a", [D, VW], F16, kind="ExternalInput").ap()
    d_bv = nc.dram_tensor("bv_a", [1, VW], F16, kind="ExternalInput").ap()
    d_wo = nc.dram_tensor("wo_t", [MLOC, D], F16, kind="ExternalInput").ap()
    d_bq = nc.dram_tensor("bq_r", [128, MTN], F32, kind="ExternalInput").ap()
    d_bk = nc.dram_tensor("bk_r", [128, MTN], F32, kind="ExternalInput").ap()
    d_on = nc.dram_tensor("ones_r", [1, 128], F16, kind="ExternalInput").ap()
    d_out = nc.dram_tensor("out_t", [D, S], F32, kind="ExternalOutput").ap()

    with tile.TileContext(nc) as tc:
        with (
            tc.tile_pool(name="persist", bufs=1) as pp,
            tc.tile_pool(name="xstream", bufs=1) as xsp,
            tc.tile_pool(name="work", bufs=1) as wk_pool,
            tc.tile_pool(name="taskp", bufs=1, space="PSUM") as tkp,
        ):
            # ---- persistent SBUF tensors ----
            wv_sb = pp.tile([128, KTN, VW], F16, tag="wv", name="wv_sb")
            nc.sync.dma_start(wv_sb[:], d_wv.rearrange("(k p) m -> p k m", p=128))
            wo_sb = pp.tile([128, MTN, D], F16, tag="wo", name="wo_sb")
            nc.sync.dma_start(wo_sb[:], d_wo.rearrange("(k p) m -> p k m", p=128))
            wq_sb = pp.tile([128, KTN, MLOC], F16, tag="wq", name="wq_sb")
            nc.sync.dma_start(wq_sb[:], d_wq.rearrange("(k p) m -> p k m", p=128))
            bv_sb = pp.tile([1, VW], F16, tag="bv", name="bv_sb")
            nc.sync.dma_start(bv_sb[:], d_bv[:])
            on_sb = pp.tile([1, 128], F16, tag="ones", name="on_sb")
            nc.sync.dma_start(on_sb[:], d_on[:])
            bq_sb = pp.tile([128, MTN], F32, tag="bq", name="bq_sb")
            nc.sync.dma_start(bq_sb[:], d_bq[:])
            bk_sb = pp.tile([128, MTN], F32, tag="bk", name="bk_sb")
            nc.sync.dma_start(bk_sb[:], d_bk[:])

            qt_ic: dict = {}
            kt = [
                pp.tile([128, S], F16, tag=f"kt{m}", name=f"kt{m}")
                for m in range(MTN)
            ]
            v_sb = [
                pp.tile([128, VW], F16, tag=f"v{st}", name=f"v{st}")
                for st in range(JTN)
            ]

            def load_x(xdram, ic):
                """8 k-tiles [128, 512] of one x^T query chunk."""
                xts = []
                for k in range(KTN):
                    xt = xsp.tile([128, ICS], F16, tag="x", bufs=10, name="xt")
                    nc.sync.dma_start(
                        xt[:],
                        xdram[k * 128 : (k + 1) * 128, ic * ICS : (ic + 1) * ICS],
                    )
                    xts.append(xt)
                return xts

            def proj_mm(dest_ap, w_sb, b_sb, xts, m, psum_pool, psum_bufs):
                ps = psum_pool.tile(
                    [128, ICS], F32, tag=psum_pool.name, bufs=psum_bufs, name="ps"
                )
                for k in range(KTN):
                    nc.tensor.matmul(
                        ps[:],
                        w_sb[:, k, m * 128 : (m + 1) * 128],
                        xts[k][:],
                        start=(k == 0),
                        stop=(k == KTN - 1),
                    )
                nc.vector.tensor_scalar_add(dest_ap, ps[:], b_sb[:, m : m + 1])

            def q_proj(ic, psum_pool, psum_bufs):
                xts = load_x(d_xq, ic)
                for m in range(MTN):
                    qtile = wk_pool.tile(
                        [128, ICS], F16, tag=f"q{m}", bufs=2, name=f"q{m}"
                    )
                    qt_ic[(ic, m)] = qtile
                    proj_mm(qtile[:], wq_sb, bq_sb, xts, m, psum_pool, psum_bufs)

            # ================= pre-phase (dense PE work) =================
            with (
                tc.tile_pool(name="wkk", bufs=1) as wkk,
                tc.tile_pool(name="prep", bufs=1, space="PSUM") as prep,
            ):
                wk_sb = wkk.tile([128, KTN, MLOC], F16, tag="wk", name="wk_sb")
                nc.sync.dma_start(wk_sb[:], d_wk.rearrange("(k p) m -> p k m", p=128))

                # K projection
                for ic in range(ICN):
                    xts = load_x(d_xk, ic)
                    for m in range(MTN):
                        proj_mm(
                            kt[m][:, ic * ICS : (ic + 1) * ICS],
                            wk_sb, bk_sb, xts, m, prep, 3,
                        )

                # V projection: x^T tiles stationary, ones row adds bias and
                # seeds the denominator columns
                for stg in range(4):
                    xvg = load_x(d_xv, stg)  # [128, 512] spanning 4 s-tiles
                    for sti in range(4):
                        st = stg * 4 + sti
                        ssl = slice(sti * 128, (sti + 1) * 128)
                        for hf in range(2):
                            pv = prep.tile(
                                [128, HW_], F32, tag="pv", bufs=3, name="pv"
                            )
                            nc.tensor.matmul(
                                pv[:],
                                on_sb[0:1, :],
                                bv_sb[0:1, hf * HW_ : (hf + 1) * HW_],
                                start=True,
                                stop=False,
                            )
                            for k in range(KTN):
                                nc.tensor.matmul(
                                    pv[:],
                                    xvg[k][:, ssl],
                                    wv_sb[:, k, hf * HW_ : (hf + 1) * HW_],
                                    start=False,
                                    stop=(k == KTN - 1),
                                )
                            nc.vector.tensor_copy(
                                v_sb[st][:, hf * HW_ : (hf + 1) * HW_], pv[:]
                            )

                # Q projection for the first query chunk
                q_proj(0, prep, 3)

            # ================= attention phase =================
            # Background PE tasks (Q-proj ic 1..3, out-proj per finished ic)
            # popped into the per-j slack of the exp-rate-limited pipeline.
            tasks = deque()
            for ic in range(1, ICN):
                tasks.append(lambda ic=ic: q_proj(ic, tkp, 2))

            yts_by_ic = {}

            def o_task(ic, fo):
                po = tkp.tile([128, ICS], F32, tag="taskp", bufs=2, name="po")
                for m in range(MTN):
                    nc.tensor.matmul(
                        po[:],
                        wo_sb[:, m, fo * 128 : (fo + 1) * 128],
                        yts_by_ic[ic][m][:],
                        start=(m == 0),
                        stop=(m == MTN - 1),
                    )
                ot = wk_pool.tile([128, ICS], F32, tag="ot", bufs=2, name="ot")
                nc.vector.tensor_copy(ot[:], po[:])
                nc.sync.dma_start(
                    d_out[fo * 128 : (fo + 1) * 128, ic * ICS : (ic + 1) * ICS],
                    ot[:],
                )

            with (
                tc.tile_pool(name="scp", bufs=1, space="PSUM") as scp,
                tc.tile_pool(name="avp", bufs=1, space="PSUM") as avp,
            ):
                pop_budget = 0.0
                for ic in range(ICN):
                    isl = slice(ic * ICS, (ic + 1) * ICS)
                    yts = []
                    for mt in range(MTN):
                        pva = avp.tile([65, ICS], F32, tag="pv", bufs=2, name="pva")
                        pvb = avp.tile([65, ICS], F32, tag="pv", bufs=2, name="pvb")

                        sc_tiles = {}

                        qcur = qt_ic[(ic, mt)]

                        def emit_scores(j, mt=mt, qcur=qcur, sc_tiles=sc_tiles):
                            jsl = slice(j * JTS, (j + 1) * JTS)
                            sc = scp.tile(
                                [128, 2 * ICS], F32, tag="sc", bufs=2, name="sc"
                            )
                            nc.tensor.matmul(
                                sc[:, 0:ICS],
                                kt[mt][0:64, jsl],
                                qcur[0:64, :],
                                start=True,
                                stop=True,
                            )
                            nc.tensor.matmul(
                                sc[:, ICS : 2 * ICS],
                                kt[mt][64:128, jsl],
                                qcur[64:128, :],
                                start=True,
                                stop=True,
                            )
                            sc_tiles[j] = sc

                        emit_scores(0)
                        emit_scores(1)
                        for j in range(JTN):
                            sc = sc_tiles.pop(j)
                            u = wk_pool.tile(
                                [128, 2 * ICS], F16, tag="u", bufs=2, name="u"
                            )
                            nc.scalar.activation(u[:], sc[:], EXP, scale=SCALE)
                            va = v_sb[j][:, (2 * mt) * 65 : (2 * mt) * 65 + 65]
                            vb = v_sb[j][
                                :, (2 * mt + 1) * 65 : (2 * mt + 1) * 65 + 65
                            ]
                            nc.tensor.matmul(
                                pva[:], va, u[:, 0:ICS],
                                start=(j == 0), stop=(j == JTN - 1),
                            )
                            nc.tensor.matmul(
                                pvb[:], vb, u[:, ICS : 2 * ICS],
                                start=(j == 0), stop=(j == JTN - 1),
                            )
                            if j + 2 < JTN:
                                emit_scores(j + 2)
                            # ~1 background task per 3 j-steps keeps PE slack used
                            pop_budget += 1.0 / 3.0
                            if pop_budget >= 1.0 and tasks:
                                pop_budget -= 1.0
                                tasks.popleft()()

                        # evacuate accumulators to SBUF right away (frees the
                        # PSUM banks for the next head pair), then normalize
                        # lazily off the critical path
                        stA = wk_pool.tile([65, ICS], F32, tag="stA", bufs=2, name="stA")
                        stB = wk_pool.tile([65, ICS], F32, tag="stB", bufs=2, name="stB")
                        nc.vector.tensor_copy(stA[:], pva[:])
                        nc.vector.tensor_copy(stB[:], pvb[:])
                        yt_t = wk_pool.tile(
                            [128, ICS], F16, tag=f"yt{mt}", bufs=2, name=f"yt{mt}"
                        )
                        dna = wk_pool.tile([1, ICS], F32, tag="dna", bufs=2, name="dna")
                        dnb = wk_pool.tile([1, ICS], F32, tag="dnb", bufs=2, name="dnb")
                        nc.vector.tensor_copy(dna[:], stA[64:65, :])
                        nc.vector.tensor_copy(dnb[:], stB[64:65, :])
                        rca = wk_pool.tile([1, ICS], F32, tag="rca", bufs=2, name="rca")
                        rcb = wk_pool.tile([1, ICS], F32, tag="rcb", bufs=2, name="rcb")
                        nc.vector.reciprocal_approx_fast(rca[:], dna[:])
                        nc.vector.reciprocal_approx_fast(rcb[:], dnb[:])
                        bca = wk_pool.tile([64, ICS], F32, tag="bc", bufs=2, name="bc")
                        bcb = wk_pool.tile([64, ICS], F32, tag="bc", bufs=2, name="bc2")
                        nc.sync.dma_start(
                            bca[:],
                            rca[0:1, :].unsqueeze(1).broadcast_to((1, 64, ICS)),
                        )
                        nc.sync.dma_start(
                            bcb[:],
                            rcb[0:1, :].unsqueeze(1).broadcast_to((1, 64, ICS)),
                        )
                        nc.vector.tensor_mul(yt_t[0:64, :], stA[0:64, :], bca[:])
                        nc.vector.tensor_mul(yt_t[64:128, :], stB[0:64, :], bcb[:])
                        yts.append(yt_t)

                    yts_by_ic[ic] = yts
                    for fo in range(KTN):
                        tasks.append(lambda ic=ic, fo=fo: o_task(ic, fo))

                while tasks:
                    tasks.popleft()()

    nc.compile()
    return nc


def _get_nc():
    if "nc" not in _CACHE:
        _CACHE["nc"] = _build()
    return _CACHE["nc"]


def _prep_core(c, query, key, value, Wq, bq, Wk, bk, Wv, bv, Wo, bo):
    b, hg = c // 2, c % 2
    fsl = slice(hg * MLOC, (hg + 1) * MLOC)
    r = lambda a: np.ascontiguousarray(a, dtype=np.float16)
    wv_a = np.zeros((D, VW), np.float32)
    bv_a = np.zeros((1, VW), np.float32)
    for h in range(HG):
        gh = hg * HG + h
        wv_a[:, h * 65 : h * 65 + 64] = Wv[gh * 64 : (gh + 1) * 64, :].T
        bv_a[0, h * 65 : h * 65 + 64] = bv[gh * 64 : (gh + 1) * 64]
        bv_a[0, h * 65 + 64] = 1.0
    return {
        "xq_t": r(query[b].T),
        "xk_t": r(key[b].T),
        "xv_t": r(value[b].T),
        "wq_t": r(Wq[fsl, :].T),
        "wk_t": r(Wk[fsl, :].T),
        "wv_a": r(wv_a),
        "bv_a": r(bv_a),
        "wo_t": r(Wo[:, fsl].T),
        "bq_r": np.ascontiguousarray(bq[fsl].reshape(MTN, 128).T, dtype=np.float32),
        "bk_r": np.ascontiguousarray(bk[fsl].reshape(MTN, 128).T, dtype=np.float32),
        "ones_r": np.ones((1, 128), np.float16),
    }


def kernel(query, key, value, Wq, bq, Wk, bk, Wv, bv, Wo, bo, _trace=False):
    query = np.asarray(query, dtype=np.float32)
    key = np.asarray(key, dtype=np.float32)
    value = np.asarray(value, dtype=np.float32)
    Wq, bq = np.asarray(Wq, np.float32), np.asarray(bq, np.float32)
    Wk, bk = np.asarray(Wk, np.float32), np.asarray(bk, np.float32)
    Wv, bv = np.asarray(Wv, np.float32), np.asarray(bv, np.float32)
    Wo, bo = np.asarray(Wo, np.float32), np.asarray(bo, np.float32)

    nc = _get_nc()
    in_maps = [
        _prep_core(c, query, key, value, Wq, bq, Wk, bk, Wv, bv, Wo, bo)
        for c in range(NC_)
    ]
    res = run_bass_kernel_spmd(nc, in_maps, core_ids=list(range(NC_)), trace=_trace)
    _CACHE["last_result"] = res

    out = np.empty((B, S, D), np.float32)
    for b in range(B):
        acc = res.results[2 * b]["out_t"].astype(np.float32) + res.results[
            2 * b + 1
        ]["out_t"].astype(np.float32)
        out[b] = acc.T + bo[None, :]
    return out


# revision 11
# speedup vs baseline: 1.4986x; 1.0226x over previous
"""Multi-head attention (B=4, S=2048, D=1024, H=16, d_k=64) on 8 TRN2 NeuronCores.

Sharding: batch (4) x head-group (2) = 8 cores. Each core computes, for its
batch b and its 8 heads:
  - column-parallel Q/K/V projections (512 output features),
  - full attention for those heads,
  - row-parallel partial output projection (contribution of its 512 features
    to all 1024 output features).
The host sums the two partial outputs per batch (Megatron row-parallel
reduction done on host) and adds the output bias.

On-chip layout (per core, f32 storage; matmuls run in float32r = fp32 with an
11-bit mantissa at full PE speed):
  Q^T, K^T : [512, 2048]  features on partitions -> K=64 scores matmuls,
                          row-packed 2 heads per 128-partition tile
  V        : [2048, 520]  8 heads x (64 values + a ones column); the ones
                          column makes the attnV matmul emit the softmax
                          denominator for free as PSUM row 64
  scores^T : [128 j, 2x512 i] PSUM; exp on ScalarE with fused 1/8 scale
  y^T      : [512, 2048]  normalized attention outputs -> out-projection

Schedule: dense PE pre-phase (K-proj, V-proj, Q-proj for the first query
chunk), then an exp-rate-limited attention pipeline (scores run 2 j-steps
ahead of attnV) with the remaining Q-proj and the output projection injected
into the per-j PE slack so ScalarE never starves.
"""

import sys
from collections import deque

for _p in ("/root/.axon_site/_ro/trn_rl_repo", "/opt/trn_rl_repo"):
    if _p not in sys.path:
        sys.path.append(_p)

import numpy as np

import concourse.bass as bass
import concourse.mybir as mybir
import concourse.tile as tile
from concourse import bacc
from concourse.bass_utils import run_bass_kernel_spmd

F32R = mybir.dt.float32r
F16 = mybir.dt.float16
F32 = mybir.dt.float32
EXP = mybir.ActivationFunctionType.Exp

B, S, D = 4, 2048, 1024
H, DK = 16, 64
HG = 8  # heads per core
MLOC = HG * DK  # 512 features per core
NC_ = 8
ICN, ICS = 4, 512  # query chunks
JTN, JTS = 16, 128  # key tiles
KTN = 8  # contraction tiles over D
MTN = 4  # head-pair tiles over MLOC
VW = HG * (DK + 1)  # 520: V row width incl. ones columns
HW_ = VW // 2  # 260, fits one PSUM bank
SCALE = 1.0 / 8.0  # 1/sqrt(DK)

_CACHE: dict = {}


def _round_f32r(a: np.ndarray) -> np.ndarray:
    """Round fp32 to the fp32r grid (11-bit mantissa; low 12 bits zero)."""
    b = np.ascontiguousarray(a, dtype=np.float32).view(np.uint32)
    b = (b + 0x800) & 0xFFFFF000
    return b.view(np.float32)


def _build():
    nc = bacc.Bacc("TRN2", target_bir_lowering=False, debug=False, num_devices=NC_)

    d_xq = nc.dram_tensor("xq_t", [D, S], F16, kind="ExternalInput").ap()
    d_xk = nc.dram_tensor("xk_t", [D, S], F16, kind="ExternalInput").ap()
    d_xv = nc.dram_tensor("xv_t", [D, S], F16, kind="ExternalInput").ap()
    d_wq = nc.dram_tensor("wq_t", [D, MLOC], F16, kind="ExternalInput").ap()
    d_wk = nc.dram_tensor("wk_t", [D, MLOC], F16, kind="ExternalInput").ap()
    d_wv = nc.dram_tensor("wv_a", [D, VW], F16, kind="ExternalInput").ap()
    d_bv = nc.dram_tensor("bv_a", [1, VW], F16, kind="ExternalInput").ap()
    d_wo = nc.dram_tensor("wo_t", [MLOC, D], F16, kind="ExternalInput").ap()
    d_bq = nc.dram_tensor("bq_r", [128, MTN], F32, kind="ExternalInput").ap()
    d_bk = nc.dram_tensor("bk_r", [128, MTN], F32, kind="ExternalInput").ap()
    d_on = nc.dram_tensor("ones_r", [1, 128], F16, kind="ExternalInput").ap()
    d_out = nc.dram_tensor("out_t", [D, S], F32, kind="ExternalOutput").ap()

    with tile.TileContext(nc) as tc:
        with (
            tc.tile_pool(name="persist", bufs=1) as pp,
            tc.tile_pool(name="xstream", bufs=1) as xsp,
            tc.tile_pool(name="work", bufs=1) as wk_pool,
            tc.tile_pool(name="taskp", bufs=1, space="PSUM") as tkp,
        ):
            # ---- persistent SBUF tensors ----
            wv_sb = pp.tile([128, KTN, VW], F16, tag="wv", name="wv_sb")
            nc.sync.dma_start(wv_sb[:], d_wv.rearrange("(k p) m -> p k m", p=128))
            wo_sb = pp.tile([128, MTN, D], F16, tag="wo", name="wo_sb")
            nc.sync.dma_start(wo_sb[:], d_wo.rearrange("(k p) m -> p k m", p=128))
            wq_sb = pp.tile([128, KTN, MLOC], F16, tag="wq", name="wq_sb")
            nc.sync.dma_start(wq_sb[:], d_wq.rearrange("(k p) m -> p k m", p=128))
            bv_sb = pp.tile([1, VW], F16, tag="bv", name="bv_sb")
            nc.sync.dma_start(bv_sb[:], d_bv[:])
            on_sb = pp.tile([1, 128], F16, tag="ones", name="on_sb")
            nc.sync.dma_start(on_sb[:], d_on[:])
            bq_sb = pp.tile([128, MTN], F32, tag="bq", name="bq_sb")
            nc.sync.dma_start(bq_sb[:], d_bq[:])
            bk_sb = pp.tile([128, MTN], F32, tag="bk", name="bk_sb")
            nc.sync.dma_start(bk_sb[:], d_bk[:])

            qt_ic: dict = {}
            kt = [
                pp.tile([128, S], F16, tag=f"kt{m}", name=f"kt{m}")
                for m in range(MTN)
            ]
            v_sb = [
                pp.tile([128, VW], F16, tag=f"v{st}", name=f"v{st}")
                for st in range(JTN)
            ]

            def load_x(xdram, ic):
                """8 k-tiles [128, 512] of one x^T query chunk."""
                xts = []
                for k in range(KTN):
                    xt = xsp.tile([128, ICS], F16, tag="x", bufs=10, name="xt")
                    nc.sync.dma_start(
                        xt[:],
                        xdram[k * 128 : (k + 1) * 128, ic * ICS : (ic + 1) * ICS],
                    )
                    xts.append(xt)
                return xts

            def proj_mm(dest_ap, w_sb, b_sb, xts, m, psum_pool, psum_bufs):
                ps = psum_pool.tile(
                    [128, ICS], F32, tag=psum_pool.name, bufs=psum_bufs, name="ps"
                )
                for k in range(KTN):
                    nc.tensor.matmul(
                        ps[:],
                        w_sb[:, k, m * 128 : (m + 1) * 128],
                        xts[k][:],
                        start=(k == 0),
                        stop=(k == KTN - 1),
                    )
                nc.vector.tensor_scalar_add(dest_ap, ps[:], b_sb[:, m : m + 1])

            def q_proj(ic, psum_pool, psum_bufs):
                xts = load_x(d_xq, ic)
                for m in range(MTN):
                    qtile = wk_pool.tile(
                        [128, ICS], F16, tag=f"q{m}", bufs=2, name=f"q{m}"
                    )
                    qt_ic[(ic, m)] = qtile
                    proj_mm(qtile[:], wq_sb, bq_sb, xts, m, psum_pool, psum_bufs)

            # ================= pre-phase (dense PE work) =================
            with (
                tc.tile_pool(name="wkk", bufs=1) as wkk,
                tc.tile_pool(name="prep", bufs=1, space="PSUM") as prep,
            ):
                wk_sb = wkk.tile([128, KTN, MLOC], F16, tag="wk", name="wk_sb")
                nc.sync.dma_start(wk_sb[:], d_wk.rearrange("(k p) m -> p k m", p=128))

                # K projection
                for ic in range(ICN):
                    xts = load_x(d_xk, ic)
                    for m in range(MTN):
                        proj_mm(
                            kt[m][:, ic * ICS : (ic + 1) * ICS],
                            wk_sb, bk_sb, xts, m, prep, 3,
                        )

                # V projection: x^T tiles stationary, ones row adds bias and
                # seeds the denominator columns
                for stg in range(4):
                    xvg = load_x(d_xv, stg)  # [128, 512] spanning 4 s-tiles
                    for sti in range(4):
                        st = stg * 4 + sti
                        ssl = slice(sti * 128, (sti + 1) * 128)
                        for hf in range(2):
                            pv = prep.tile(
                                [128, HW_], F32, tag="pv", bufs=3, name="pv"
                            )
                            nc.tensor.matmul(
                                pv[:],
                                on_sb[0:1, :],
                                bv_sb[0:1, hf * HW_ : (hf + 1) * HW_],
                                start=True,
                                stop=False,
                            )
                            for k in range(KTN):
                                nc.tensor.matmul(
                                    pv[:],
                                    xvg[k][:, ssl],
                                    wv_sb[:, k, hf * HW_ : (hf + 1) * HW_],
                                    start=False,
                                    stop=(k == KTN - 1),
                                )
                            nc.vector.tensor_copy(
                                v_sb[st][:, hf * HW_ : (hf + 1) * HW_], pv[:]
                            )

                # Q projection for the first query chunk
                q_proj(0, prep, 3)

            # ================= attention phase =================
            # Background PE tasks (Q-proj ic 1..3, out-proj per finished ic)
            # popped into the per-j slack of the exp-rate-limited pipeline.
            tasks = deque()
            for ic in range(1, ICN):
                tasks.append(lambda ic=ic: q_proj(ic, tkp, 2))

            yts_by_ic = {}

            def o_task(ic, fo):
                po = tkp.tile([128, ICS], F32, tag="taskp", bufs=2, name="po")
                for m in range(MTN):
                    nc.tensor.matmul(
                        po[:],
                        wo_sb[:, m, fo * 128 : (fo + 1) * 128],
                        yts_by_ic[ic][m][:],
                        start=(m == 0),
                        stop=(m == MTN - 1),
                    )
                ot = wk_pool.tile([128, ICS], F32, tag="ot", bufs=2, name="ot")
                nc.vector.tensor_copy(ot[:], po[:])
                nc.sync.dma_start(
                    d_out[fo * 128 : (fo + 1) * 128, ic * ICS : (ic + 1) * ICS],
                    ot[:],
                )

            with (
                tc.tile_pool(name="scp", bufs=1, space="PSUM") as scp,
                tc.tile_pool(name="avp", bufs=1, space="PSUM") as avp,
            ):
                pop_budget = 0.0
                for ic in range(ICN):
                    isl = slice(ic * ICS, (ic + 1) * ICS)
                    yts = []
                    for mt in range(MTN):
                        pva = avp.tile([65, ICS], F32, tag="pv", bufs=2, name="pva")
                        pvb = avp.tile([65, ICS], F32, tag="pv", bufs=2, name="pvb")

                        sc_tiles = {}

                        qcur = qt_ic[(ic, mt)]

                        def emit_scores(j, mt=mt, qcur=qcur, sc_tiles=sc_tiles):
                            jsl = slice(j * JTS, (j + 1) * JTS)
                            sc = scp.tile(
                                [128, 2 * ICS], F32, tag="sc", bufs=2, name="sc"
                            )
                            nc.tensor.matmul(
                                sc[:, 0:ICS],
                                kt[mt][0:64, jsl],
                                qcur[0:64, :],
                                start=True,
                                stop=True,
                            )
                            nc.tensor.matmul(
                                sc[:, ICS : 2 * ICS],
                                kt[mt][64:128, jsl],
                                qcur[64:128, :],
                                start=True,
                                stop=True,
                            )
                            sc_tiles[j] = sc

                        emit_scores(0)
                        emit_scores(1)
                        for j in range(JTN):
                            sc = sc_tiles.pop(j)
                            u = wk_pool.tile(
                                [128, 2 * ICS], F16, tag="u", bufs=2, name="u"
                            )
                            nc.scalar.activation(u[:], sc[:], EXP, scale=SCALE)
                            va = v_sb[j][:, (2 * mt) * 65 : (2 * mt) * 65 + 65]
                            vb = v_sb[j][
                                :, (2 * mt + 1) * 65 : (2 * mt + 1) * 65 + 65
                            ]
                            nc.tensor.matmul(
                                pva[:], va, u[:, 0:ICS],
                                start=(j == 0), stop=(j == JTN - 1),
                            )
                            nc.tensor.matmul(
                                pvb[:], vb, u[:, ICS : 2 * ICS],
                                start=(j == 0), stop=(j == JTN - 1),
                            )
                            if j + 2 < JTN:
                                emit_scores(j + 2)
                            # ~1 background task per 3 j-steps keeps PE slack used
                            pop_budget += 1.0 / 3.0
                            if pop_budget >= 1.0 and tasks:
                                pop_budget -= 1.0
                                tasks.popleft()()

                        # evacuate accumulators to SBUF right away (frees the
                        # PSUM banks for the next head pair), then normalize
                        # lazily off the critical path
                        stA = wk_pool.tile([65, ICS], F32, tag="stA", bufs=2, name="stA")
                        stB = wk_pool.tile([65, ICS], F32, tag="stB", bufs=2, name="stB")
                        nc.vector.tensor_copy(stA[:], pva[:])
                        nc.vector.tensor_copy(stB[:], pvb[:])
                        yt_t = wk_pool.tile(
                            [128, ICS], F16, tag=f"yt{mt}", bufs=2, name=f"yt{mt}"
                        )
                        dna = wk_pool.tile([1, ICS], F32, tag="dna", bufs=2, name="dna")
                        dnb = wk_pool.tile([1, ICS], F32, tag="dnb", bufs=2, name="dnb")
                        nc.vector.tensor_copy(dna[:], stA[64:65, :])
                        nc.vector.tensor_copy(dnb[:], stB[64:65, :])
                        rca = wk_pool.tile([1, ICS], F32, tag="rca", bufs=2, name="rca")
                        rcb = wk_pool.tile([1, ICS], F32, tag="rcb", bufs=2, name="rcb")
                        nc.vector.reciprocal_approx_fast(rca[:], dna[:])
                        nc.vector.reciprocal_approx_fast(rcb[:], dnb[:])
                        bca = wk_pool.tile([64, ICS], F32, tag="bc", bufs=2, name="bc")
                        bcb = wk_pool.tile([64, ICS], F32, tag="bc", bufs=2, name="bc2")
                        nc.gpsimd.dma_start(
                            bca[:],
                            rca[0:1, :].unsqueeze(1).broadcast_to((1, 64, ICS)),
                        )
                        nc.gpsimd.dma_start(
                            bcb[:],
                            rcb[0:1, :].unsqueeze(1).broadcast_to((1, 64, ICS)),
                        )
                        nc.vector.tensor_mul(yt_t[0:64, :], stA[0:64, :], bca[:])
                        nc.vector.tensor_mul(yt_t[64:128, :], stB[0:64, :], bcb[:])
                        yts.append(yt_t)

                    yts_by_ic[ic] = yts
                    for fo in range(KTN):
                        tasks.append(lambda ic=ic, fo=fo: o_task(ic, fo))

                while tasks:
                    tasks.popleft()()

    nc.compile()
    return nc


def _get_nc():
    if "nc" not in _CACHE:
        _CACHE["nc"] = _build()
    return _CACHE["nc"]


def _prep_core(c, query, key, value, Wq, bq, Wk, bk, Wv, bv, Wo, bo):
    b, hg = c // 2, c % 2
    fsl = slice(hg * MLOC, (hg + 1) * MLOC)
    r = lambda a: np.ascontiguousarray(a, dtype=np.float16)
    wv_a = np.zeros((D, VW), np.float32)
    bv_a = np.zeros((1, VW), np.float32)
    for h in range(HG):
        gh = hg * HG + h
        wv_a[:, h * 65 : h * 65 + 64] = Wv[gh * 64 : (gh + 1) * 64, :].T
        bv_a[0, h * 65 : h * 65 + 64] = bv[gh * 64 : (gh + 1) * 64]
        bv_a[0, h * 65 + 64] = 1.0
    return {
        "xq_t": r(query[b].T),
        "xk_t": r(key[b].T),
        "xv_t": r(value[b].T),
        "wq_t": r(Wq[fsl, :].T),
        "wk_t": r(Wk[fsl, :].T),
        "wv_a": r(wv_a),
        "bv_a": r(bv_a),
        "wo_t": r(Wo[:, fsl].T),
        "bq_r": np.ascontiguousarray(bq[fsl].reshape(MTN, 128).T, dtype=np.float32),
        "bk_r": np.ascontiguousarray(bk[fsl].reshape(MTN, 128).T, dtype=np.float32),
        "ones_r": np.ones((1, 128), np.float16),
    }


def kernel(query, key, value, Wq, bq, Wk, bk, Wv, bv, Wo, bo, _trace=False):
    query = np.asarray(query, dtype=np.float32)
    key = np.asarray(key, dtype=np.float32)
    value = np.asarray(value, dtype=np.float32)
    Wq, bq = np.asarray(Wq, np.float32), np.asarray(bq, np.float32)
    Wk, bk = np.asarray(Wk, np.float32), np.asarray(bk, np.float32)
    Wv, bv = np.asarray(Wv, np.float32), np.asarray(bv, np.float32)
    Wo, bo = np.asarray(Wo, np.float32), np.asarray(bo, np.float32)

    nc = _get_nc()
    in_maps = [
        _prep_core(c, query, key, value, Wq, bq, Wk, bk, Wv, bv, Wo, bo)
        for c in range(NC_)
    ]
    res = run_bass_kernel_spmd(nc, in_maps, core_ids=list(range(NC_)), trace=_trace)
    _CACHE["last_result"] = res

    out = np.empty((B, S, D), np.float32)
    for b in range(B):
        acc = res.results[2 * b]["out_t"].astype(np.float32) + res.results[
            2 * b + 1
        ]["out_t"].astype(np.float32)
        out[b] = acc.T + bo[None, :]
    return out
